# revision 6
# baseline (speedup 1.0000x reference)
"""Trainium2 Bass kernel for MHA (B=2, T=2048, D=1024, H=16, HD=64).

Sharding: tensor-parallel over heads. Each of 8 cores handles 2 heads
(a 128-row slice of Wq/Wk/Wv, 128-column slice of Wo), for both batch
elements:
  - QKV: x is PE-transposed to x^T once, then Q^T/K^T/V^T [128, 4096]
    (head dims on partitions) via per-core weight slices.
  - Attention per batch (flash-style, no max subtraction -- scores are
    O(1) by construction): S^T[k,q] tiles with d=64 contraction, the two
    heads row-packed on the PE array; exp on ScalarE (softmax scale
    fused into the activation's free affine); PV with an extra ones
    column in V so the softmax denominator falls out of the same
    matmuls.
  - Softmax division commutes with fc_out, so it is deferred: fc_out is
    split per head (row-packed K=64 matmul pairs), each half scaled by
    its head's per-token reciprocal (a per-partition tensor_scalar),
    halves summed on GpSimd.
  - The 8 partial outputs are summed on the host (the all-reduce of the
    row-sharded fc_out happens at gather time); bo is added on host.

All matmuls run in float32r (TF32-like, ~1e-4 rel err, 4x faster than
fp32 on the PE); accumulation is fp32 in PSUM.
"""

import sys

sys.path.insert(0, "/opt/trn_rl_repo")

import numpy as np

import concourse.bass as bass
import concourse.mybir as mybir
import concourse.tile as tile
from concourse import bacc
from concourse.bass_utils import run_bass_kernel_spmd
from concourse.masks import make_identity

DT = mybir.dt
AF = mybir.ActivationFunctionType

B, T, D, H, HD = 2, 2048, 1024, 16, 64
NTOK = B * T              # 4096
NCORES = 8
OSL = D // NCORES         # 128 output dims per core (2 heads)
TCH = 512                 # QKV token chunk
NCH = NTOK // TCH         # 8
QC = 1024                 # attention q chunk
KTILES = T // 128         # 16 k tiles per batch
SCALE = 1.0 / np.sqrt(HD)

F32 = DT.float32
F32R = DT.float32r


def build_nc():
    nc = bacc.Bacc("TRN2", target_bir_lowering=False, debug=False)

    x_d = nc.dram_tensor("x", [NTOK, D], F32, kind="ExternalInput")
    wq_d = nc.dram_tensor("wq", [OSL, D], F32, kind="ExternalInput")
    wk_d = nc.dram_tensor("wk", [OSL, D], F32, kind="ExternalInput")
    wv_d = nc.dram_tensor("wv", [OSL, D], F32, kind="ExternalInput")
    bq_d = nc.dram_tensor("bq", [OSL], F32, kind="ExternalInput")
    bk_d = nc.dram_tensor("bk", [OSL], F32, kind="ExternalInput")
    bv_d = nc.dram_tensor("bv", [OSL], F32, kind="ExternalInput")
    wo_d = nc.dram_tensor("wo", [D, OSL], F32, kind="ExternalInput")
    out_d = nc.dram_tensor("out", [NTOK, D], F32, kind="ExternalOutput")

    with tile.TileContext(nc) as tc:
        with tc.tile_pool(name="persist", bufs=1) as pp:
            # ---- constants ----
            ident = pp.tile([128, 128], F32, tag="ident")
            make_identity(nc, ident[:])

            # ---- weights prep: W slices -> transposed fp32r tiles ----
            wt = {}
            with tc.tile_pool(name="prep", bufs=2) as prep, \
                 tc.tile_pool(name="prep_ps", bufs=2, space="PSUM") as prep_ps:
                for name, w_d in (("q", wq_d), ("k", wk_d), ("v", wv_d)):
                    w_nat = prep.tile([128, D], F32, tag="wnat")
                    nc.sync.dma_start(w_nat[:], w_d[:, :])
                    wt_t = pp.tile([128, 8, 128], F32R, tag=f"wt_{name}")
                    for it in range(8):
                        ps = prep_ps.tile([128, 128], F32, tag="wps")
                        nc.tensor.transpose(ps[:], w_nat[:, it * 128:(it + 1) * 128], ident[:])
                        nc.any.tensor_copy(wt_t[:, it, :], ps[:])
                    wt[name] = wt_t
                # Wo slice [D, OSL]: natural [o2-part, d] -> WoT [d-part, o2]
                wo_nat = prep.tile([128, 8, OSL], F32, tag="wonat")
                nc.sync.dma_start(wo_nat[:], wo_d.rearrange("(a p) d -> p a d", p=128))
                woT = pp.tile([128, D], F32R, tag="woT")
                for it in range(8):
                    ps = prep_ps.tile([128, 128], F32, tag="wps")
                    nc.tensor.transpose(ps[:], wo_nat[:, it, :], ident[:])
                    nc.any.tensor_copy(woT[:, it * 128:(it + 1) * 128], ps[:])

            btiles = {}
            for name, b_d in (("q", bq_d), ("k", bk_d), ("v", bv_d)):
                bt = pp.tile([128, 1], F32, tag=f"b_{name}")
                nc.sync.dma_start(bt[:], b_d[:, None])
                btiles[name] = bt

            # ---- persistent activations ----
            qT = pp.tile([128, NTOK], F32R, tag="qT")
            kT = pp.tile([128, NTOK], F32R, tag="kT")
            # V natural with ones column per t-tile: 32 slots of [128, 130]
            vaug = pp.tile([128, 32 * 130], F32R, tag="vaug")
            oT = pp.tile([128, NTOK], F32R, tag="oT")
            # per-token softmax reciprocal, token-partition layout, per head
            recipA = pp.tile([128, 32], F32, tag="recipA")
            recipB = pp.tile([128, 32], F32, tag="recipB")
            # reciprocal broadcast across head-dim partitions [d, q]
            bcs = pp.tile([128, NTOK], F32, tag="bcs")
            ones_row = pp.tile([1, 128], F32, tag="ones_row")
            nc.vector.memset(ones_row[:], 1.0)

            # ones columns of vaug (col 64 = head A, col 129 = head B);
            # memset doesn't support fp32r, so copy from an fp32 ones tile
            ones2 = pp.tile([128, 2], F32, tag="ones2")
            nc.vector.memset(ones2[:], 1.0)
            for ti in range(32):
                nc.vector.tensor_copy(
                    vaug[:, ti * 130 + 64:ti * 130 + 130:65], ones2[:])

            # ================= Phase 1: QKV =================
            with tc.tile_pool(name="ph1", bufs=2) as ph1, \
                 tc.tile_pool(name="ph1_ps", bufs=2, space="PSUM") as ph1_ps, \
                 tc.tile_pool(name="ph1_ps3", bufs=3, space="PSUM") as ph1_ps3:
                for ch in range(NCH):
                    t0 = ch * TCH
                    x_nat = ph1.tile([128, 4, D], F32, tag="xnat")
                    nc.sync.dma_start(
                        x_nat[:], x_d[t0:t0 + TCH, :].rearrange("(a p) i -> p a i", p=128))
                    xT = ph1.tile([128, 8, TCH], F32R, tag="xT")
                    for it in range(8):
                        ps = ph1_ps.tile([128, 512], F32, tag="xtp")
                        for a in range(4):
                            nc.tensor.transpose(
                                ps[:, a * 128:(a + 1) * 128],
                                x_nat[:, a, it * 128:(it + 1) * 128], ident[:])
                        nc.any.tensor_copy(xT[:, it, :], ps[:])
                    for name in ("q", "k", "v"):
                        psq = ph1_ps3.tile([128, TCH], F32, tag="qkvp")
                        for it in range(8):
                            nc.tensor.matmul(
                                psq[:], wt[name][:, it, :], xT[:, it, :],
                                start=(it == 0), stop=(it == 7))
                        if name == "q":
                            nc.any.tensor_scalar_add(qT[:, t0:t0 + TCH], psq[:], btiles[name][:])
                        elif name == "k":
                            nc.any.tensor_scalar_add(kT[:, t0:t0 + TCH], psq[:], btiles[name][:])
                        else:
                            vT = ph1.tile([128, TCH], F32, tag="vT")
                            nc.any.tensor_scalar_add(vT[:], psq[:], btiles[name][:])
                            # transpose V^T [d, t] -> V natural [t, d] into vaug
                            psv = ph1_ps.tile([128, 4, 128], F32, tag="vtp")
                            for a in range(4):
                                nc.tensor.transpose(
                                    psv[:, a, :], vT[:, a * 128:(a + 1) * 128], ident[:])
                            ti0 = ch * 4  # first of the 4 t-tiles in this chunk
                            va_view = vaug[:, ti0 * 130:(ti0 + 4) * 130].rearrange(
                                "p (a c) -> p a c", c=130)
                            nc.any.tensor_copy(va_view[:, :, 0:64], psv[:, :, 0:64])
                            nc.any.tensor_copy(va_view[:, :, 65:129], psv[:, :, 64:128])

            # ================= Phase 2: attention =================
            with tc.tile_pool(name="ph2", bufs=2) as ph2, \
                 tc.tile_pool(name="att_ps", bufs=1, space="PSUM") as att_ps:
                for b in range(B):
                    for qc in range(T // QC):
                        q0 = b * T + qc * QC
                        otA = att_ps.tile([65, QC], F32, tag="otA")
                        otB = att_ps.tile([65, QC], F32, tag="otB")
                        for kt in range(KTILES):
                            k0 = b * T + kt * 128
                            stA = att_ps.tile([128, QC], F32, tag="stA")
                            stB = att_ps.tile([128, QC], F32, tag="stB")
                            for qq in range(QC // 512):
                                qsl = slice(q0 + qq * 512, q0 + (qq + 1) * 512)
                                osl = slice(qq * 512, (qq + 1) * 512)
                                nc.tensor.matmul(
                                    stA[:, osl], kT[0:64, k0:k0 + 128], qT[0:64, qsl],
                                    start=True, stop=True)
                                nc.tensor.matmul(
                                    stB[:, osl], kT[64:128, k0:k0 + 128], qT[64:128, qsl],
                                    start=True, stop=True)
                            pTA = ph2.tile([128, QC], F32R, tag="pTA")
                            pTB = ph2.tile([128, QC], F32R, tag="pTB")
                            nc.scalar.activation(pTA[:], stA[:], AF.Exp, scale=float(SCALE))
                            nc.scalar.activation(pTB[:], stB[:], AF.Exp, scale=float(SCALE))
                            ti = (b * T + kt * 128) // 128
                            for qq in range(QC // 512):
                                osl = slice(qq * 512, (qq + 1) * 512)
                                nc.tensor.matmul(
                                    otA[:, osl],
                                    vaug[:, ti * 130 + 0:ti * 130 + 65], pTA[:, osl],
                                    start=(kt == 0), stop=(kt == KTILES - 1))
                                nc.tensor.matmul(
                                    otB[:, osl],
                                    vaug[:, ti * 130 + 65:ti * 130 + 130], pTB[:, osl],
                                    start=(kt == 0), stop=(kt == KTILES - 1))
                        # unnormalized head outputs -> SBUF
                        nc.any.tensor_copy(oT[0:64, q0:q0 + QC], otA[0:64, :])
                        nc.any.tensor_copy(oT[64:128, q0:q0 + QC], otB[0:64, :])
                        # softmax denominators (psum row 64) -> SBUF rows
                        sumsA = ph2.tile([1, QC], F32, tag="sumsA")
                        sumsB = ph2.tile([1, QC], F32, tag="sumsB")
                        nc.vector.tensor_copy(sumsA[:], otA[64:65, :])
                        nc.vector.tensor_copy(sumsB[:], otB[64:65, :])
                        # transpose to token-partition layout, reciprocal
                        tt0 = q0 // 128
                        rcA = att_ps.tile([128, 8], F32, tag="stA")
                        rcB = att_ps.tile([128, 8], F32, tag="stB")
                        for i in range(QC // 128):
                            nc.tensor.transpose(
                                rcA[:, i:i + 1], sumsA[0:1, i * 128:(i + 1) * 128],
                                ident[0:1, 0:1])
                            nc.tensor.transpose(
                                rcB[:, i:i + 1], sumsB[0:1, i * 128:(i + 1) * 128],
                                ident[0:1, 0:1])
                        nc.vector.reciprocal(recipA[:, tt0:tt0 + 8], rcA[:])
                        nc.vector.reciprocal(recipB[:, tt0:tt0 + 8], rcB[:])

            # ===== Phase 2b: broadcast reciprocals to [d, q] layout =====
            # recip8 [128, 32] (token-partition) -> PE transpose -> [32, 128]
            # -> DMA gather to a [1, 4096] row -> K=1 fp32 matmul against a
            # ones column broadcasts it across the 64 head-dim partitions.
            with tc.tile_pool(name="ph2b", bufs=2) as ph2b, \
                 tc.tile_pool(name="bc_ps", bufs=2, space="PSUM") as bc_ps:
                rows = {}
                for name, rc8 in (("A", recipA), ("B", recipB)):
                    rps = bc_ps.tile([32, 128], F32, tag="rback")
                    nc.tensor.transpose(rps[:], rc8[:], ident[:])
                    rb = ph2b.tile([32, 128], F32, tag="rb")
                    nc.vector.tensor_copy(rb[:], rps[:])
                    row = ph2b.tile([1, NTOK], F32, tag=f"row{name}")
                    nc.sync.dma_start(row[:], rb[:])
                    rows[name] = row
                for c8 in range(NTOK // 512):
                    csl = slice(c8 * 512, (c8 + 1) * 512)
                    bps = bc_ps.tile([128, 512], F32, tag="bcp")
                    nc.tensor.matmul(bps[0:64, :], ones_row[0:1, 0:64],
                                     rows["A"][0:1, csl], start=True, stop=True)
                    nc.tensor.matmul(bps[64:128, :], ones_row[0:1, 0:64],
                                     rows["B"][0:1, csl], start=True, stop=True)
                    nc.any.tensor_copy(bcs[:, csl], bps[:])

            # ================= Phase 3: fc_out =================
            with tc.tile_pool(name="ph3", bufs=3) as ph3, \
                 tc.tile_pool(name="fc_ps", bufs=2, space="PSUM") as fc_ps:
                for tt in range(NTOK // 128):
                    tsl = slice(tt * 128, (tt + 1) * 128)
                    oTn = ph3.tile([128, 128], F32R, tag="oTn")
                    nc.vector.tensor_tensor(oTn[:], oT[:, tsl], bcs[:, tsl],
                                            mybir.AluOpType.mult)
                    for oc in range(D // 512):
                        owsl = slice(oc * 512, (oc + 1) * 512)
                        psf = fc_ps.tile([128, 512], F32, tag="fcp")
                        nc.tensor.matmul(psf[:], oTn[:], woT[:, owsl],
                                         start=True, stop=True)
                        fcs = ph3.tile([128, 512], F32, tag="fcs")
                        nc.any.tensor_copy(fcs[:], psf[:])
                        nc.sync.dma_start(
                            out_d[tt * 128:(tt + 1) * 128, owsl], fcs[:])

    nc.compile()
    return nc


_NC = None


def _get_nc():
    global _NC
    if _NC is None:
        _NC = build_nc()
    return _NC


def kernel(**inputs):
    x = np.ascontiguousarray(np.asarray(inputs["x"], dtype=np.float32).reshape(NTOK, D))
    Wq = np.asarray(inputs["Wq"], dtype=np.float32)
    Wk = np.asarray(inputs["Wk"], dtype=np.float32)
    Wv = np.asarray(inputs["Wv"], dtype=np.float32)
    Wo = np.asarray(inputs["Wo"], dtype=np.float32)
    bq = np.asarray(inputs["bq"], dtype=np.float32)
    bk = np.asarray(inputs["bk"], dtype=np.float32)
    bv = np.asarray(inputs["bv"], dtype=np.float32)
    bo = np.asarray(inputs["bo"], dtype=np.float32)

    nc = _get_nc()
    in_maps = []
    for c in range(NCORES):
        sl = slice(c * OSL, (c + 1) * OSL)
        in_maps.append({
            "x": x,
            "wq": np.ascontiguousarray(Wq[sl, :]),
            "wk": np.ascontiguousarray(Wk[sl, :]),
            "wv": np.ascontiguousarray(Wv[sl, :]),
            "bq": np.ascontiguousarray(bq[sl]),
            "bk": np.ascontiguousarray(bk[sl]),
            "bv": np.ascontiguousarray(bv[sl]),
            "wo": np.ascontiguousarray(Wo[:, sl]),
        })
    res = run_bass_kernel_spmd(nc, in_maps, core_ids=list(range(NCORES)))
    acc = np.zeros((NTOK, D), dtype=np.float64)
    for c in range(NCORES):
        acc += res.results[c]["out"]
    acc += bo.astype(np.float64)[None, :]
    return acc.astype(np.float32).reshape(B, T, D)


# revision 7
# speedup vs baseline: 1.0028x; 1.0028x over previous
"""Trainium2 Bass kernel for MHA (B=2, T=2048, D=1024, H=16, HD=64).

Sharding: tensor-parallel over heads. Each of 8 cores handles 2 heads
(a 128-row slice of Wq/Wk/Wv, 128-column slice of Wo), for both batch
elements:
  - QKV: x is PE-transposed to x^T once, then Q^T/K^T/V^T [128, 4096]
    (head dims on partitions) via per-core weight slices.
  - Attention per batch (flash-style, no max subtraction -- scores are
    O(1) by construction): S^T[k,q] tiles with d=64 contraction, the two
    heads row-packed on the PE array; exp on ScalarE (softmax scale
    fused into the activation's free affine); PV with an extra ones
    column in V so the softmax denominator falls out of the same
    matmuls.
  - Softmax division commutes with fc_out, so it is deferred: fc_out is
    split per head (row-packed K=64 matmul pairs), each half scaled by
    its head's per-token reciprocal (a per-partition tensor_scalar),
    halves summed on GpSimd.
  - The 8 partial outputs are summed on the host (the all-reduce of the
    row-sharded fc_out happens at gather time); bo is added on host.

All matmuls run in float32r (TF32-like, ~1e-4 rel err, 4x faster than
fp32 on the PE); accumulation is fp32 in PSUM.
"""

import sys

sys.path.insert(0, "/opt/trn_rl_repo")

import numpy as np

import concourse.bass as bass
import concourse.mybir as mybir
import concourse.tile as tile
from concourse import bacc
from concourse.bass_utils import run_bass_kernel_spmd
from concourse.masks import make_identity

DT = mybir.dt
AF = mybir.ActivationFunctionType

B, T, D, H, HD = 2, 2048, 1024, 16, 64
NTOK = B * T              # 4096
NCORES = 8
OSL = D // NCORES         # 128 output dims per core (2 heads)
TCH = 512                 # QKV token chunk
NCH = NTOK // TCH         # 8
QC = 1024                 # attention q chunk
KTILES = T // 128         # 16 k tiles per batch
SCALE = 1.0 / np.sqrt(HD)

F32 = DT.float32
F32R = DT.float32r


def build_nc():
    nc = bacc.Bacc("TRN2", target_bir_lowering=False, debug=False)

    x_d = nc.dram_tensor("x", [NTOK, D], F32, kind="ExternalInput")
    wq_d = nc.dram_tensor("wq", [OSL, D], F32, kind="ExternalInput")
    wk_d = nc.dram_tensor("wk", [OSL, D], F32, kind="ExternalInput")
    wv_d = nc.dram_tensor("wv", [OSL, D], F32, kind="ExternalInput")
    bq_d = nc.dram_tensor("bq", [OSL], F32, kind="ExternalInput")
    bk_d = nc.dram_tensor("bk", [OSL], F32, kind="ExternalInput")
    bv_d = nc.dram_tensor("bv", [OSL], F32, kind="ExternalInput")
    wo_d = nc.dram_tensor("wo", [D, OSL], F32, kind="ExternalInput")
    out_d = nc.dram_tensor("out", [NTOK, D], F32, kind="ExternalOutput")

    with tile.TileContext(nc) as tc:
        with tc.tile_pool(name="persist", bufs=1) as pp:
            # ---- constants ----
            ident = pp.tile([128, 128], F32, tag="ident")
            make_identity(nc, ident[:])

            # ---- weights prep: W slices -> transposed fp32r tiles ----
            wt = {}
            with tc.tile_pool(name="prep", bufs=2) as prep, \
                 tc.tile_pool(name="prep_ps", bufs=2, space="PSUM") as prep_ps:
                for name, w_d in (("q", wq_d), ("k", wk_d), ("v", wv_d)):
                    w_nat = prep.tile([128, D], F32, tag="wnat")
                    nc.sync.dma_start(w_nat[:], w_d[:, :])
                    wt_t = pp.tile([128, 8, 128], F32R, tag=f"wt_{name}")
                    for it in range(8):
                        ps = prep_ps.tile([128, 128], F32, tag="wps")
                        nc.tensor.transpose(ps[:], w_nat[:, it * 128:(it + 1) * 128], ident[:])
                        nc.any.tensor_copy(wt_t[:, it, :], ps[:])
                    wt[name] = wt_t
                # Wo slice [D, OSL]: natural [o2-part, d] -> WoT [d-part, o2]
                wo_nat = prep.tile([128, 8, OSL], F32, tag="wonat")
                nc.sync.dma_start(wo_nat[:], wo_d.rearrange("(a p) d -> p a d", p=128))
                woT = pp.tile([128, D], F32R, tag="woT")
                for it in range(8):
                    ps = prep_ps.tile([128, 128], F32, tag="wps")
                    nc.tensor.transpose(ps[:], wo_nat[:, it, :], ident[:])
                    nc.any.tensor_copy(woT[:, it * 128:(it + 1) * 128], ps[:])

            btiles = {}
            for name, b_d in (("q", bq_d), ("k", bk_d), ("v", bv_d)):
                bt = pp.tile([128, 1], F32, tag=f"b_{name}")
                nc.sync.dma_start(bt[:], b_d[:, None])
                btiles[name] = bt

            # ---- persistent activations ----
            qT = pp.tile([128, NTOK], F32R, tag="qT")
            kT = pp.tile([128, NTOK], F32R, tag="kT")
            # V natural with ones column per t-tile: 32 slots of [128, 130]
            vaug = pp.tile([128, 32 * 130], F32R, tag="vaug")
            oT = pp.tile([128, NTOK], F32R, tag="oT")
            # per-token softmax reciprocal, token-partition layout, per head
            recipA = pp.tile([128, 32], F32, tag="recipA")
            recipB = pp.tile([128, 32], F32, tag="recipB")
            # reciprocal broadcast across head-dim partitions [d, q]
            bcs = pp.tile([128, NTOK], F32, tag="bcs")
            ones_row = pp.tile([1, 128], F32, tag="ones_row")
            nc.vector.memset(ones_row[:], 1.0)

            # ones columns of vaug (col 64 = head A, col 129 = head B);
            # memset doesn't support fp32r, so copy from an fp32 ones tile
            ones2 = pp.tile([128, 2], F32, tag="ones2")
            nc.vector.memset(ones2[:], 1.0)
            for ti in range(32):
                nc.vector.tensor_copy(
                    vaug[:, ti * 130 + 64:ti * 130 + 130:65], ones2[:])

            # ================= Phase 1: QKV =================
            with tc.tile_pool(name="ph1", bufs=2) as ph1, \
                 tc.tile_pool(name="ph1_ps", bufs=2, space="PSUM") as ph1_ps, \
                 tc.tile_pool(name="ph1_ps3", bufs=3, space="PSUM") as ph1_ps3:
                for ch in range(NCH):
                    t0 = ch * TCH
                    x_nat = ph1.tile([128, 4, D], F32, tag="xnat")
                    nc.sync.dma_start(
                        x_nat[:], x_d[t0:t0 + TCH, :].rearrange("(a p) i -> p a i", p=128))
                    xT = ph1.tile([128, 8, TCH], F32R, tag="xT")
                    for it in range(8):
                        ps = ph1_ps.tile([128, 512], F32, tag="xtp")
                        for a in range(4):
                            nc.tensor.transpose(
                                ps[:, a * 128:(a + 1) * 128],
                                x_nat[:, a, it * 128:(it + 1) * 128], ident[:])
                        nc.any.tensor_copy(xT[:, it, :], ps[:])
                    for name in ("q", "k", "v"):
                        psq = ph1_ps3.tile([128, TCH], F32, tag="qkvp")
                        for it in range(8):
                            nc.tensor.matmul(
                                psq[:], wt[name][:, it, :], xT[:, it, :],
                                start=(it == 0), stop=(it == 7))
                        if name == "q":
                            nc.any.tensor_scalar_add(qT[:, t0:t0 + TCH], psq[:], btiles[name][:])
                        elif name == "k":
                            nc.any.tensor_scalar_add(kT[:, t0:t0 + TCH], psq[:], btiles[name][:])
                        else:
                            vT = ph1.tile([128, TCH], F32, tag="vT")
                            nc.any.tensor_scalar_add(vT[:], psq[:], btiles[name][:])
                            # transpose V^T [d, t] -> V natural [t, d] into vaug
                            psv = ph1_ps.tile([128, 4, 128], F32, tag="vtp")
                            for a in range(4):
                                nc.tensor.transpose(
                                    psv[:, a, :], vT[:, a * 128:(a + 1) * 128], ident[:])
                            ti0 = ch * 4  # first of the 4 t-tiles in this chunk
                            va_view = vaug[:, ti0 * 130:(ti0 + 4) * 130].rearrange(
                                "p (a c) -> p a c", c=130)
                            nc.any.tensor_copy(va_view[:, :, 0:64], psv[:, :, 0:64])
                            nc.any.tensor_copy(va_view[:, :, 65:129], psv[:, :, 64:128])

            # ================= Phase 2: attention =================
            with tc.tile_pool(name="ph2", bufs=2) as ph2, \
                 tc.tile_pool(name="att_ps", bufs=1, space="PSUM") as att_ps:
                for b in range(B):
                    for qc in range(T // QC):
                        q0 = b * T + qc * QC
                        otA = att_ps.tile([65, QC], F32, tag="otA")
                        otB = att_ps.tile([65, QC], F32, tag="otB")
                        for kt in range(KTILES):
                            k0 = b * T + kt * 128
                            stA = att_ps.tile([128, QC], F32, tag="stA")
                            stB = att_ps.tile([128, QC], F32, tag="stB")
                            for qq in range(QC // 512):
                                qsl = slice(q0 + qq * 512, q0 + (qq + 1) * 512)
                                osl = slice(qq * 512, (qq + 1) * 512)
                                nc.tensor.matmul(
                                    stA[:, osl], kT[0:64, k0:k0 + 128], qT[0:64, qsl],
                                    start=True, stop=True, tile_position=(0, 0))
                                nc.tensor.matmul(
                                    stB[:, osl], kT[64:128, k0:k0 + 128], qT[64:128, qsl],
                                    start=True, stop=True, tile_position=(64, 0))
                            pTA = ph2.tile([128, QC], F32R, tag="pTA")
                            pTB = ph2.tile([128, QC], F32R, tag="pTB")
                            nc.scalar.activation(pTA[:], stA[:], AF.Exp, scale=float(SCALE))
                            nc.scalar.activation(pTB[:], stB[:], AF.Exp, scale=float(SCALE))
                            ti = (b * T + kt * 128) // 128
                            for qq in range(QC // 512):
                                osl = slice(qq * 512, (qq + 1) * 512)
                                nc.tensor.matmul(
                                    otA[:, osl],
                                    vaug[:, ti * 130 + 0:ti * 130 + 65], pTA[:, osl],
                                    start=(kt == 0), stop=(kt == KTILES - 1))
                                nc.tensor.matmul(
                                    otB[:, osl],
                                    vaug[:, ti * 130 + 65:ti * 130 + 130], pTB[:, osl],
                                    start=(kt == 0), stop=(kt == KTILES - 1))
                        # unnormalized head outputs -> SBUF
                        nc.any.tensor_copy(oT[0:64, q0:q0 + QC], otA[0:64, :])
                        nc.any.tensor_copy(oT[64:128, q0:q0 + QC], otB[0:64, :])
                        # softmax denominators (psum row 64) -> SBUF rows
                        sumsA = ph2.tile([1, QC], F32, tag="sumsA")
                        sumsB = ph2.tile([1, QC], F32, tag="sumsB")
                        nc.vector.tensor_copy(sumsA[:], otA[64:65, :])
                        nc.vector.tensor_copy(sumsB[:], otB[64:65, :])
                        # transpose to token-partition layout, reciprocal
                        tt0 = q0 // 128
                        rcA = att_ps.tile([128, 8], F32, tag="stA")
                        rcB = att_ps.tile([128, 8], F32, tag="stB")
                        for i in range(QC // 128):
                            nc.tensor.transpose(
                                rcA[:, i:i + 1], sumsA[0:1, i * 128:(i + 1) * 128],
                                ident[0:1, 0:1])
                            nc.tensor.transpose(
                                rcB[:, i:i + 1], sumsB[0:1, i * 128:(i + 1) * 128],
                                ident[0:1, 0:1])
                        nc.vector.reciprocal(recipA[:, tt0:tt0 + 8], rcA[:])
                        nc.vector.reciprocal(recipB[:, tt0:tt0 + 8], rcB[:])

            # ===== Phase 2b: broadcast reciprocals to [d, q] layout =====
            # recip8 [128, 32] (token-partition) -> PE transpose -> [32, 128]
            # -> DMA gather to a [1, 4096] row -> K=1 fp32 matmul against a
            # ones column broadcasts it across the 64 head-dim partitions.
            with tc.tile_pool(name="ph2b", bufs=2) as ph2b, \
                 tc.tile_pool(name="bc_ps", bufs=2, space="PSUM") as bc_ps:
                rows = {}
                for name, rc8 in (("A", recipA), ("B", recipB)):
                    rps = bc_ps.tile([32, 128], F32, tag="rback")
                    nc.tensor.transpose(rps[:], rc8[:], ident[:])
                    rb = ph2b.tile([32, 128], F32, tag="rb")
                    nc.vector.tensor_copy(rb[:], rps[:])
                    row = ph2b.tile([1, NTOK], F32, tag=f"row{name}")
                    nc.sync.dma_start(row[:], rb[:])
                    rows[name] = row
                for c8 in range(NTOK // 512):
                    csl = slice(c8 * 512, (c8 + 1) * 512)
                    bps = bc_ps.tile([128, 512], F32, tag="bcp")
                    nc.tensor.matmul(bps[0:64, :], ones_row[0:1, 0:64],
                                     rows["A"][0:1, csl], start=True, stop=True)
                    nc.tensor.matmul(bps[64:128, :], ones_row[0:1, 0:64],
                                     rows["B"][0:1, csl], start=True, stop=True)
                    nc.any.tensor_copy(bcs[:, csl], bps[:])

            # ================= Phase 3: fc_out =================
            with tc.tile_pool(name="ph3", bufs=3) as ph3, \
                 tc.tile_pool(name="fc_ps", bufs=2, space="PSUM") as fc_ps:
                for tt in range(NTOK // 128):
                    tsl = slice(tt * 128, (tt + 1) * 128)
                    oTn = ph3.tile([128, 128], F32R, tag="oTn")
                    nc.vector.tensor_tensor(oTn[:], oT[:, tsl], bcs[:, tsl],
                                            mybir.AluOpType.mult)
                    for oc in range(D // 512):
                        owsl = slice(oc * 512, (oc + 1) * 512)
                        psf = fc_ps.tile([128, 512], F32, tag="fcp")
                        nc.tensor.matmul(psf[:], oTn[:], woT[:, owsl],
                                         start=True, stop=True)
                        fcs = ph3.tile([128, 512], F32, tag="fcs")
                        nc.any.tensor_copy(fcs[:], psf[:])
                        nc.sync.dma_start(
                            out_d[tt * 128:(tt + 1) * 128, owsl], fcs[:])

    nc.compile()
    return nc


_NC = None


def _get_nc():
    global _NC
    if _NC is None:
        _NC = build_nc()
    return _NC


def kernel(**inputs):
    x = np.ascontiguousarray(np.asarray(inputs["x"], dtype=np.float32).reshape(NTOK, D))
    Wq = np.asarray(inputs["Wq"], dtype=np.float32)
    Wk = np.asarray(inputs["Wk"], dtype=np.float32)
    Wv = np.asarray(inputs["Wv"], dtype=np.float32)
    Wo = np.asarray(inputs["Wo"], dtype=np.float32)
    bq = np.asarray(inputs["bq"], dtype=np.float32)
    bk = np.asarray(inputs["bk"], dtype=np.float32)
    bv = np.asarray(inputs["bv"], dtype=np.float32)
    bo = np.asarray(inputs["bo"], dtype=np.float32)

    nc = _get_nc()
    in_maps = []
    for c in range(NCORES):
        sl = slice(c * OSL, (c + 1) * OSL)
        in_maps.append({
            "x": x,
            "wq": np.ascontiguousarray(Wq[sl, :]),
            "wk": np.ascontiguousarray(Wk[sl, :]),
            "wv": np.ascontiguousarray(Wv[sl, :]),
            "bq": np.ascontiguousarray(bq[sl]),
            "bk": np.ascontiguousarray(bk[sl]),
            "bv": np.ascontiguousarray(bv[sl]),
            "wo": np.ascontiguousarray(Wo[:, sl]),
        })
    res = run_bass_kernel_spmd(nc, in_maps, core_ids=list(range(NCORES)))
    acc = np.zeros((NTOK, D), dtype=np.float64)
    for c in range(NCORES):
        acc += res.results[c]["out"]
    acc += bo.astype(np.float64)[None, :]
    return acc.astype(np.float32).reshape(B, T, D)


# revision 9
# speedup vs baseline: 1.2520x; 1.2486x over previous
"""Trainium2 Bass kernel for MHA (B=2, T=2048, D=1024, H=16, HD=64).

Sharding: tensor-parallel over heads. Each of 8 cores handles 2 heads
(a 128-row slice of Wq/Wk/Wv, 128-column slice of Wo), for both batch
elements:
  - QKV: x is PE-transposed to x^T once, then Q^T/K^T/V^T [128, 4096]
    (head dims on partitions) via per-core weight slices.
  - Attention per batch (flash-style, no max subtraction -- scores are
    O(1) by construction): S^T[k,q] tiles with d=64 contraction, the two
    heads row-packed on the PE array; exp on ScalarE (softmax scale
    fused into the activation's free affine); PV with an extra ones
    column in V so the softmax denominator falls out of the same
    matmuls.
  - Softmax division commutes with fc_out, so it is deferred: fc_out is
    split per head (row-packed K=64 matmul pairs), each half scaled by
    its head's per-token reciprocal (a per-partition tensor_scalar),
    halves summed on GpSimd.
  - The 8 partial outputs are summed on the host (the all-reduce of the
    row-sharded fc_out happens at gather time); bo is added on host.

All matmuls run in float32r (TF32-like, ~1e-4 rel err, 4x faster than
fp32 on the PE); accumulation is fp32 in PSUM.
"""

import sys

sys.path.insert(0, "/opt/trn_rl_repo")

import numpy as np

import concourse.bass as bass
import concourse.mybir as mybir
import concourse.tile as tile
from concourse import bacc
from concourse.bass_utils import run_bass_kernel_spmd
from concourse.masks import make_identity

DT = mybir.dt
AF = mybir.ActivationFunctionType

B, T, D, H, HD = 2, 2048, 1024, 16, 64
NTOK = B * T              # 4096
NCORES = 8
OSL = D // NCORES         # 128 output dims per core (2 heads)
TCH = 512                 # QKV token chunk
NCH = NTOK // TCH         # 8
QC = 512                  # attention q chunk
KTILES = T // 128         # 16 k tiles per batch
SCALE = 1.0 / np.sqrt(HD)

F32 = DT.float32
F32R = DT.float32r


def build_nc():
    nc = bacc.Bacc("TRN2", target_bir_lowering=False, debug=False)

    x_d = nc.dram_tensor("x", [NTOK, D], F32, kind="ExternalInput")
    wq_d = nc.dram_tensor("wq", [OSL, D], F32, kind="ExternalInput")
    wk_d = nc.dram_tensor("wk", [OSL, D], F32, kind="ExternalInput")
    wv_d = nc.dram_tensor("wv", [OSL, D], F32, kind="ExternalInput")
    bq_d = nc.dram_tensor("bq", [OSL], F32, kind="ExternalInput")
    bk_d = nc.dram_tensor("bk", [OSL], F32, kind="ExternalInput")
    bv_d = nc.dram_tensor("bv", [OSL], F32, kind="ExternalInput")
    wo_d = nc.dram_tensor("wo", [D, OSL], F32, kind="ExternalInput")
    out_d = nc.dram_tensor("out", [NTOK, D], F32, kind="ExternalOutput")

    with tile.TileContext(nc) as tc:
        with tc.tile_pool(name="persist", bufs=1) as pp:
            # ---- constants ----
            ident = pp.tile([128, 128], F32, tag="ident")
            make_identity(nc, ident[:])

            # ---- weights prep: W slices -> transposed fp32r tiles ----
            wt = {}
            with tc.tile_pool(name="prep", bufs=2) as prep, \
                 tc.tile_pool(name="prep_ps", bufs=2, space="PSUM") as prep_ps:
                for name, w_d in (("q", wq_d), ("k", wk_d), ("v", wv_d)):
                    w_nat = prep.tile([128, D], F32, tag="wnat")
                    nc.sync.dma_start(w_nat[:], w_d[:, :])
                    wt_t = pp.tile([128, 8, 128], F32R, tag=f"wt_{name}")
                    for it in range(8):
                        ps = prep_ps.tile([128, 128], F32, tag="wps")
                        nc.tensor.transpose(ps[:], w_nat[:, it * 128:(it + 1) * 128], ident[:])
                        nc.any.tensor_copy(wt_t[:, it, :], ps[:])
                    wt[name] = wt_t
                # Wo slice [D, OSL]: natural [o2-part, d] -> WoT [d-part, o2]
                wo_nat = prep.tile([128, 8, OSL], F32, tag="wonat")
                nc.sync.dma_start(wo_nat[:], wo_d.rearrange("(a p) d -> p a d", p=128))
                woT = pp.tile([128, D], F32R, tag="woT")
                for it in range(8):
                    ps = prep_ps.tile([128, 128], F32, tag="wps")
                    nc.tensor.transpose(ps[:], wo_nat[:, it, :], ident[:])
                    nc.any.tensor_copy(woT[:, it * 128:(it + 1) * 128], ps[:])

            btiles = {}
            for name, b_d in (("q", bq_d), ("k", bk_d), ("v", bv_d)):
                bt = pp.tile([128, 1], F32, tag=f"b_{name}")
                nc.sync.dma_start(bt[:], b_d[:, None])
                btiles[name] = bt

            # ---- persistent activations ----
            qT = pp.tile([128, NTOK], F32R, tag="qT")
            kT = pp.tile([128, NTOK], F32R, tag="kT")
            # V natural with ones column per t-tile: 32 slots of [128, 130]
            vaug = pp.tile([128, 32 * 130], F32R, tag="vaug")
            oT = pp.tile([128, NTOK], F32R, tag="oT")
            # per-token softmax reciprocal, token-partition layout, per head
            recipA = pp.tile([128, 32], F32, tag="recipA")
            recipB = pp.tile([128, 32], F32, tag="recipB")
            # reciprocal broadcast across head-dim partitions [d, q]
            bcs = pp.tile([128, NTOK], F32, tag="bcs")
            ones_row = pp.tile([1, 128], F32, tag="ones_row")
            nc.vector.memset(ones_row[:], 1.0)

            # ones columns of vaug (col 64 = head A, col 129 = head B);
            # memset doesn't support fp32r, so copy from an fp32 ones tile
            ones2 = pp.tile([128, 2], F32, tag="ones2")
            nc.vector.memset(ones2[:], 1.0)
            for ti in range(32):
                nc.vector.tensor_copy(
                    vaug[:, ti * 130 + 64:ti * 130 + 130:65], ones2[:])

            # ================= Phase 1: QKV =================
            with tc.tile_pool(name="ph1", bufs=2) as ph1, \
                 tc.tile_pool(name="ph1_ps", bufs=2, space="PSUM") as ph1_ps, \
                 tc.tile_pool(name="ph1_ps3", bufs=3, space="PSUM") as ph1_ps3:
                for ch in range(NCH):
                    t0 = ch * TCH
                    x_nat = ph1.tile([128, 4, D], F32, tag="xnat")
                    nc.sync.dma_start(
                        x_nat[:], x_d[t0:t0 + TCH, :].rearrange("(a p) i -> p a i", p=128))
                    xT = ph1.tile([128, 8, TCH], F32R, tag="xT")
                    for it in range(8):
                        ps = ph1_ps.tile([128, 512], F32, tag="xtp")
                        for a in range(4):
                            nc.tensor.transpose(
                                ps[:, a * 128:(a + 1) * 128],
                                x_nat[:, a, it * 128:(it + 1) * 128], ident[:])
                        nc.any.tensor_copy(xT[:, it, :], ps[:])
                    for name in ("q", "k", "v"):
                        psq = ph1_ps3.tile([128, TCH], F32, tag="qkvp")
                        for it in range(8):
                            nc.tensor.matmul(
                                psq[:], wt[name][:, it, :], xT[:, it, :],
                                start=(it == 0), stop=(it == 7))
                        if name == "q":
                            nc.any.tensor_scalar_add(qT[:, t0:t0 + TCH], psq[:], btiles[name][:])
                        elif name == "k":
                            nc.any.tensor_scalar_add(kT[:, t0:t0 + TCH], psq[:], btiles[name][:])
                        else:
                            vT = ph1.tile([128, TCH], F32, tag="vT")
                            nc.any.tensor_scalar_add(vT[:], psq[:], btiles[name][:])
                            # transpose V^T [d, t] -> V natural [t, d] into vaug
                            psv = ph1_ps.tile([128, 4, 128], F32, tag="vtp")
                            for a in range(4):
                                nc.tensor.transpose(
                                    psv[:, a, :], vT[:, a * 128:(a + 1) * 128], ident[:])
                            ti0 = ch * 4  # first of the 4 t-tiles in this chunk
                            va_view = vaug[:, ti0 * 130:(ti0 + 4) * 130].rearrange(
                                "p (a c) -> p a c", c=130)
                            nc.any.tensor_copy(va_view[:, :, 0:64], psv[:, :, 0:64])
                            nc.any.tensor_copy(va_view[:, :, 65:129], psv[:, :, 64:128])

            # ================= Phase 2: attention =================
            # q-chunks of 512; S^T for both heads lives in ONE [128, 1024]
            # psum tile (head A cols 0:512, head B cols 512:1024) so the
            # row-packed S matmul pair shares a single wait and the exp
            # covers both heads in one F=1024 activation.
            with tc.tile_pool(name="ph2", bufs=3) as ph2, \
                 tc.tile_pool(name="att_ps", bufs=1, space="PSUM") as att_ps, \
                 tc.tile_pool(name="st_ps", bufs=2, space="PSUM") as st_ps:
                for b in range(B):
                    for qc in range(T // QC):
                        q0 = b * T + qc * QC
                        otA = att_ps.tile([65, QC], F32, tag="otA")
                        otB = att_ps.tile([65, QC], F32, tag="otB")
                        for kt in range(KTILES):
                            k0 = b * T + kt * 128
                            stAB = st_ps.tile([128, 2 * QC], F32, tag="stAB")
                            nc.tensor.matmul(
                                stAB[:, 0:QC], kT[0:64, k0:k0 + 128],
                                qT[0:64, q0:q0 + QC],
                                start=True, stop=True, tile_position=(0, 0))
                            nc.tensor.matmul(
                                stAB[:, QC:2 * QC], kT[64:128, k0:k0 + 128],
                                qT[64:128, q0:q0 + QC],
                                start=True, stop=True, tile_position=(64, 0))
                            pTAB = ph2.tile([128, 2 * QC], F32R, tag="pTAB")
                            nc.scalar.activation(pTAB[:], stAB[:], AF.Exp, scale=float(SCALE))
                            ti = (b * T + kt * 128) // 128
                            nc.tensor.matmul(
                                otA[:, :], vaug[:, ti * 130 + 0:ti * 130 + 65],
                                pTAB[:, 0:QC],
                                start=(kt == 0), stop=(kt == KTILES - 1))
                            nc.tensor.matmul(
                                otB[:, :], vaug[:, ti * 130 + 65:ti * 130 + 130],
                                pTAB[:, QC:2 * QC],
                                start=(kt == 0), stop=(kt == KTILES - 1))
                        # unnormalized head outputs -> SBUF
                        nc.any.tensor_copy(oT[0:64, q0:q0 + QC], otA[0:64, :])
                        nc.any.tensor_copy(oT[64:128, q0:q0 + QC], otB[0:64, :])
                        # softmax denominators (psum row 64) -> SBUF rows
                        sumsA = ph2.tile([1, QC], F32, tag="sumsA")
                        sumsB = ph2.tile([1, QC], F32, tag="sumsB")
                        nc.vector.tensor_copy(sumsA[:], otA[64:65, :])
                        nc.vector.tensor_copy(sumsB[:], otB[64:65, :])
                        # transpose to token-partition layout, reciprocal
                        tt0 = q0 // 128
                        rcA = att_ps.tile([128, QC // 128], F32, tag="otA")
                        rcB = att_ps.tile([128, QC // 128], F32, tag="otB")
                        for i in range(QC // 128):
                            nc.tensor.transpose(
                                rcA[:, i:i + 1], sumsA[0:1, i * 128:(i + 1) * 128],
                                ident[0:1, 0:1])
                            nc.tensor.transpose(
                                rcB[:, i:i + 1], sumsB[0:1, i * 128:(i + 1) * 128],
                                ident[0:1, 0:1])
                        nc.vector.reciprocal(recipA[:, tt0:tt0 + QC // 128], rcA[:])
                        nc.vector.reciprocal(recipB[:, tt0:tt0 + QC // 128], rcB[:])

            # ===== Phase 2b: broadcast reciprocals to [d, q] layout =====
            # recip8 [128, 32] (token-partition) -> PE transpose -> [32, 128]
            # -> DMA gather to a [1, 4096] row -> K=1 fp32 matmul against a
            # ones column broadcasts it across the 64 head-dim partitions.
            with tc.tile_pool(name="ph2b", bufs=2) as ph2b, \
                 tc.tile_pool(name="bc_ps", bufs=2, space="PSUM") as bc_ps:
                rows = {}
                for name, rc8 in (("A", recipA), ("B", recipB)):
                    rps = bc_ps.tile([32, 128], F32, tag="rback")
                    nc.tensor.transpose(rps[:], rc8[:], ident[:])
                    rb = ph2b.tile([32, 128], F32, tag="rb")
                    nc.vector.tensor_copy(rb[:], rps[:])
                    row = ph2b.tile([1, NTOK], F32, tag=f"row{name}")
                    nc.sync.dma_start(row[:], rb[:])
                    rows[name] = row
                for c8 in range(NTOK // 512):
                    csl = slice(c8 * 512, (c8 + 1) * 512)
                    bps = bc_ps.tile([128, 512], F32, tag="bcp")
                    nc.tensor.matmul(bps[0:64, :], ones_row[0:1, 0:64],
                                     rows["A"][0:1, csl], start=True, stop=True)
                    nc.tensor.matmul(bps[64:128, :], ones_row[0:1, 0:64],
                                     rows["B"][0:1, csl], start=True, stop=True)
                    nc.any.tensor_copy(bcs[:, csl], bps[:])

            # ================= Phase 3: fc_out =================
            with tc.tile_pool(name="ph3", bufs=3) as ph3, \
                 tc.tile_pool(name="fc_ps", bufs=2, space="PSUM") as fc_ps:
                for tt in range(NTOK // 128):
                    tsl = slice(tt * 128, (tt + 1) * 128)
                    oTn = ph3.tile([128, 128], F32R, tag="oTn")
                    nc.vector.tensor_tensor(oTn[:], oT[:, tsl], bcs[:, tsl],
                                            mybir.AluOpType.mult)
                    for oc in range(D // 512):
                        owsl = slice(oc * 512, (oc + 1) * 512)
                        psf = fc_ps.tile([128, 512], F32, tag="fcp")
                        nc.tensor.matmul(psf[:], oTn[:], woT[:, owsl],
                                         start=True, stop=True)
                        fcs = ph3.tile([128, 512], F32, tag="fcs")
                        nc.any.tensor_copy(fcs[:], psf[:])
                        nc.sync.dma_start(
                            out_d[tt * 128:(tt + 1) * 128, owsl], fcs[:])

    nc.compile()
    return nc


_NC = None


def _get_nc():
    global _NC
    if _NC is None:
        _NC = build_nc()
    return _NC


def kernel(**inputs):
    x = np.ascontiguousarray(np.asarray(inputs["x"], dtype=np.float32).reshape(NTOK, D))
    Wq = np.asarray(inputs["Wq"], dtype=np.float32)
    Wk = np.asarray(inputs["Wk"], dtype=np.float32)
    Wv = np.asarray(inputs["Wv"], dtype=np.float32)
    Wo = np.asarray(inputs["Wo"], dtype=np.float32)
    bq = np.asarray(inputs["bq"], dtype=np.float32)
    bk = np.asarray(inputs["bk"], dtype=np.float32)
    bv = np.asarray(inputs["bv"], dtype=np.float32)
    bo = np.asarray(inputs["bo"], dtype=np.float32)

    nc = _get_nc()
    in_maps = []
    for c in range(NCORES):
        sl = slice(c * OSL, (c + 1) * OSL)
        in_maps.append({
            "x": x,
            "wq": np.ascontiguousarray(Wq[sl, :]),
            "wk": np.ascontiguousarray(Wk[sl, :]),
            "wv": np.ascontiguousarray(Wv[sl, :]),
            "bq": np.ascontiguousarray(bq[sl]),
            "bk": np.ascontiguousarray(bk[sl]),
            "bv": np.ascontiguousarray(bv[sl]),
            "wo": np.ascontiguousarray(Wo[:, sl]),
        })
    res = run_bass_kernel_spmd(nc, in_maps, core_ids=list(range(NCORES)))
    acc = np.zeros((NTOK, D), dtype=np.float64)
    for c in range(NCORES):
        acc += res.results[c]["out"]
    acc += bo.astype(np.float64)[None, :]
    return acc.astype(np.float32).reshape(B, T, D)


# revision 12
# speedup vs baseline: 1.3459x; 1.0750x over previous
"""Trainium2 Bass kernel for MHA (B=2, T=2048, D=1024, H=16, HD=64).

Sharding: tensor-parallel over heads. Each of 8 cores handles 2 heads
(a 128-row slice of Wq/Wk/Wv, 128-column slice of Wo), for both batch
elements:
  - QKV: x is PE-transposed to x^T once, then Q^T/K^T/V^T [128, 4096]
    (head dims on partitions) via per-core weight slices.
  - Attention per batch (flash-style, no max subtraction -- scores are
    O(1) by construction): S^T[k,q] tiles with d=64 contraction, the two
    heads row-packed on the PE array; exp on ScalarE (softmax scale
    fused into the activation's free affine); PV with an extra ones
    column in V so the softmax denominator falls out of the same
    matmuls.
  - Softmax division commutes with fc_out, so it is deferred: fc_out is
    split per head (row-packed K=64 matmul pairs), each half scaled by
    its head's per-token reciprocal (a per-partition tensor_scalar),
    halves summed on GpSimd.
  - The 8 partial outputs are summed on the host (the all-reduce of the
    row-sharded fc_out happens at gather time); bo is added on host.

All matmuls run in float32r (TF32-like, ~1e-4 rel err, 4x faster than
fp32 on the PE); accumulation is fp32 in PSUM.
"""

import sys

sys.path.insert(0, "/opt/trn_rl_repo")

import numpy as np

import concourse.bass as bass
import concourse.mybir as mybir
import concourse.tile as tile
from concourse import bacc
from concourse.bass_utils import run_bass_kernel_spmd
from concourse.masks import make_identity

DT = mybir.dt
AF = mybir.ActivationFunctionType

B, T, D, H, HD = 2, 2048, 1024, 16, 64
NTOK = B * T              # 4096
NCORES = 8
OSL = D // NCORES         # 128 output dims per core (2 heads)
TCH = 512                 # QKV token chunk
NCH = NTOK // TCH         # 8
QC = 512                  # attention q chunk
KTILES = T // 128         # 16 k tiles per batch
SCALE = 1.0 / np.sqrt(HD)

F32 = DT.float32
F32R = DT.float32r


def build_nc():
    nc = bacc.Bacc("TRN2", target_bir_lowering=False, debug=False)

    x_d = nc.dram_tensor("x", [NTOK, D], F32, kind="ExternalInput")
    wq_d = nc.dram_tensor("wq", [OSL, D], F32, kind="ExternalInput")
    wk_d = nc.dram_tensor("wk", [OSL, D], F32, kind="ExternalInput")
    wv_d = nc.dram_tensor("wv", [OSL, D], F32, kind="ExternalInput")
    bq_d = nc.dram_tensor("bq", [OSL], F32, kind="ExternalInput")
    bk_d = nc.dram_tensor("bk", [OSL], F32, kind="ExternalInput")
    bv_d = nc.dram_tensor("bv", [OSL], F32, kind="ExternalInput")
    wo_d = nc.dram_tensor("wo", [D, OSL], F32, kind="ExternalInput")
    out_d = nc.dram_tensor("out", [NTOK, D], F32, kind="ExternalOutput")

    with tile.TileContext(nc) as tc:
        with tc.tile_pool(name="persist", bufs=1) as pp:
            # ---- constants ----
            ident = pp.tile([128, 128], F32, tag="ident")
            make_identity(nc, ident[:])
            identr = pp.tile([128, 128], F32R, tag="identr")
            nc.vector.tensor_copy(identr[:], ident[:])

            # ---- weights prep: W slices -> transposed fp32r tiles ----
            wt = {}
            with tc.tile_pool(name="prep", bufs=2) as prep, \
                 tc.tile_pool(name="prep_ps", bufs=2, space="PSUM") as prep_ps:
                for name, w_d in (("q", wq_d), ("k", wk_d), ("v", wv_d)):
                    w_nat = prep.tile([128, D], F32, tag="wnat")
                    nc.sync.dma_start(w_nat[:], w_d[:, :])
                    wt_t = pp.tile([128, 8, 128], F32R, tag=f"wt_{name}")
                    for it in range(8):
                        ps = prep_ps.tile([128, 128], F32, tag="wps")
                        nc.tensor.transpose(ps[:], w_nat[:, it * 128:(it + 1) * 128], ident[:])
                        nc.vector.tensor_copy(wt_t[:, it, :], ps[:])
                    wt[name] = wt_t
                # Wo slice [D, OSL]: natural [o2-part, d] -> WoT [d-part, o2]
                wo_nat = prep.tile([128, 8, OSL], F32, tag="wonat")
                nc.sync.dma_start(wo_nat[:], wo_d.rearrange("(a p) d -> p a d", p=128))
                woT = pp.tile([128, D], F32R, tag="woT")
                for it in range(8):
                    ps = prep_ps.tile([128, 128], F32, tag="wps")
                    nc.tensor.transpose(ps[:], wo_nat[:, it, :], ident[:])
                    nc.vector.tensor_copy(woT[:, it * 128:(it + 1) * 128], ps[:])

            btiles = {}
            for name, b_d in (("q", bq_d), ("k", bk_d), ("v", bv_d)):
                bt = pp.tile([128, 1], F32, tag=f"b_{name}")
                nc.sync.dma_start(bt[:], b_d[:, None])
                btiles[name] = bt

            # ---- persistent activations ----
            qT = pp.tile([128, NTOK], F32R, tag="qT")
            kT = pp.tile([128, NTOK], F32R, tag="kT")
            # V natural with ones column per t-tile: 32 slots of [128, 130]
            vaug = pp.tile([128, 32 * 130], F32R, tag="vaug")
            oT = pp.tile([128, NTOK], F32R, tag="oT")
            # per-token softmax reciprocal, token-partition layout, per head
            recipA = pp.tile([128, 32], F32, tag="recipA")
            recipB = pp.tile([128, 32], F32, tag="recipB")
            # reciprocal broadcast across head-dim partitions [d, q]
            bcs = pp.tile([128, NTOK], F32, tag="bcs")

            # ones columns of vaug (col 64 = head A, col 129 = head B);
            # memset doesn't support fp32r, so copy from an fp32 ones tile
            ones2 = pp.tile([128, 2], F32, tag="ones2")
            nc.vector.memset(ones2[:], 1.0)
            for ti in range(32):
                nc.vector.tensor_copy(
                    vaug[:, ti * 130 + 64:ti * 130 + 130:65], ones2[:])

            # ================= Phase 1: QKV =================
            with tc.tile_pool(name="ph1", bufs=2) as ph1, \
                 tc.tile_pool(name="ph1_ps", bufs=2, space="PSUM") as ph1_ps, \
                 tc.tile_pool(name="ph1_ps3", bufs=3, space="PSUM") as ph1_ps3:
                for ch in range(NCH):
                    t0 = ch * TCH
                    x_nat = ph1.tile([128, 4, D], F32R, tag="xnat")
                    nc.gpsimd.dma_start(
                        x_nat[:], x_d[t0:t0 + TCH, :].rearrange("(a p) i -> p a i", p=128))
                    xT = ph1.tile([128, 8, TCH], F32R, tag="xT")
                    for it in range(8):
                        ps = ph1_ps.tile([128, 512], F32R, tag="xtp")
                        for a in range(4):
                            nc.tensor.transpose(
                                ps[:, a * 128:(a + 1) * 128],
                                x_nat[:, a, it * 128:(it + 1) * 128], identr[:])
                        nc.scalar.copy(xT[:, it, :], ps[:])
                    for name in ("q", "k", "v"):
                        psq = ph1_ps3.tile([128, TCH], F32, tag="qkvp")
                        for it in range(8):
                            nc.tensor.matmul(
                                psq[:], wt[name][:, it, :], xT[:, it, :],
                                start=(it == 0), stop=(it == 7))
                        if name == "q":
                            nc.vector.tensor_scalar_add(qT[:, t0:t0 + TCH], psq[:], btiles[name][:])
                        elif name == "k":
                            nc.vector.tensor_scalar_add(kT[:, t0:t0 + TCH], psq[:], btiles[name][:])
                        else:
                            vT = ph1.tile([128, TCH], F32R, tag="vT")
                            nc.vector.tensor_scalar_add(vT[:], psq[:], btiles[name][:])
                            # transpose V^T [d, t] -> V natural [t, d] into vaug
                            psv = ph1_ps.tile([128, 4, 128], F32R, tag="vtp")
                            for a in range(4):
                                nc.tensor.transpose(
                                    psv[:, a, :], vT[:, a * 128:(a + 1) * 128], identr[:])
                            ti0 = ch * 4  # first of the 4 t-tiles in this chunk
                            va_view = vaug[:, ti0 * 130:(ti0 + 4) * 130].rearrange(
                                "p (a c) -> p a c", c=130)
                            nc.vector.tensor_copy(va_view[:, :, 0:64], psv[:, :, 0:64])
                            nc.vector.tensor_copy(va_view[:, :, 65:129], psv[:, :, 64:128])

            # ======== Phase 2: attention + fc_out, per batch ========
            # q-chunks of 512; S^T for both heads lives in ONE [128, 1024]
            # psum tile (head A cols 0:512, head B cols 512:1024) so the
            # row-packed S matmul pair shares a single wait and the exp
            # covers both heads in one F=1024 activation. fc_out for batch
            # b overlaps attention for batch b+1 (8 PSUM banks total).
            with tc.tile_pool(name="ph2", bufs=3) as ph2, \
                 tc.tile_pool(name="att_ps", bufs=1, space="PSUM") as att_ps, \
                 tc.tile_pool(name="st_ps", bufs=2, space="PSUM") as st_ps, \
                 tc.tile_pool(name="fc_ps", bufs=2, space="PSUM") as fc_ps, \
                 tc.tile_pool(name="dramp", bufs=2, space="DRAM") as dramp:
                for b in range(B):
                    for qc in range(T // QC):
                        q0 = b * T + qc * QC
                        otA = att_ps.tile([65, QC], F32, tag="otA")
                        otB = att_ps.tile([65, QC], F32, tag="otB")
                        for kt in range(KTILES):
                            k0 = b * T + kt * 128
                            stAB = st_ps.tile([128, 2 * QC], F32, tag="stAB")
                            nc.tensor.matmul(
                                stAB[:, 0:QC], kT[0:64, k0:k0 + 128],
                                qT[0:64, q0:q0 + QC],
                                start=True, stop=True, tile_position=(0, 0))
                            nc.tensor.matmul(
                                stAB[:, QC:2 * QC], kT[64:128, k0:k0 + 128],
                                qT[64:128, q0:q0 + QC],
                                start=True, stop=True, tile_position=(64, 0))
                            pTAB = ph2.tile([128, 2 * QC], F32R, tag="pTAB")
                            nc.scalar.activation(pTAB[:], stAB[:], AF.Exp, scale=float(SCALE))
                            ti = (b * T + kt * 128) // 128
                            nc.tensor.matmul(
                                otA[:, :], vaug[:, ti * 130 + 0:ti * 130 + 65],
                                pTAB[:, 0:QC],
                                start=(kt == 0), stop=(kt == KTILES - 1))
                            nc.tensor.matmul(
                                otB[:, :], vaug[:, ti * 130 + 65:ti * 130 + 130],
                                pTAB[:, QC:2 * QC],
                                start=(kt == 0), stop=(kt == KTILES - 1))
                        # unnormalized head outputs -> SBUF
                        nc.vector.tensor_copy(oT[0:64, q0:q0 + QC], otA[0:64, :])
                        nc.vector.tensor_copy(oT[64:128, q0:q0 + QC], otB[0:64, :])
                        # softmax denominators (psum row 64) -> SBUF rows
                        sumsA = ph2.tile([1, QC], F32, tag="sumsA")
                        sumsB = ph2.tile([1, QC], F32, tag="sumsB")
                        nc.vector.tensor_copy(sumsA[:], otA[64:65, :])
                        nc.vector.tensor_copy(sumsB[:], otB[64:65, :])
                        # transpose to token-partition layout, reciprocal
                        tt0 = q0 // 128
                        rcA = att_ps.tile([128, QC // 128], F32, tag="otA")
                        rcB = att_ps.tile([128, QC // 128], F32, tag="otB")
                        for i in range(QC // 128):
                            nc.tensor.transpose(
                                rcA[:, i:i + 1], sumsA[0:1, i * 128:(i + 1) * 128],
                                ident[0:1, 0:1])
                            nc.tensor.transpose(
                                rcB[:, i:i + 1], sumsB[0:1, i * 128:(i + 1) * 128],
                                ident[0:1, 0:1])
                        nc.vector.reciprocal(recipA[:, tt0:tt0 + QC // 128], rcA[:])
                        nc.vector.reciprocal(recipB[:, tt0:tt0 + QC // 128], rcB[:])

                    # -- batch b reciprocal broadcast: recip [128, 16] -> PE
                    # transpose -> [16, 128] -> DRAM row -> partition-broadcast
                    # DMA into bcs rows (per head) --
                    bsl = slice(b * T, (b + 1) * T)
                    for hname, rc8, psl in (("A", recipA, slice(0, 64)),
                                            ("B", recipB, slice(64, 128))):
                        rps = att_ps.tile([16, 128], F32, tag="otA")
                        nc.tensor.transpose(rps[:], rc8[:, b * 16:(b + 1) * 16], ident[:])
                        rb = ph2.tile([16, 128], F32, tag="rb")
                        nc.vector.tensor_copy(rb[:], rps[:])
                        row_d = dramp.tile([1, T], F32, tag=f"row{hname}")
                        nc.sync.dma_start(row_d[:], rb[:])
                        nc.sync.dma_start(bcs[psl, bsl],
                                          row_d[0:1, :].to_broadcast([64, T]))

                    # -- fc_out for batch b --
                    for tt in range(b * 16, (b + 1) * 16):
                        tsl = slice(tt * 128, (tt + 1) * 128)
                        oTn = ph2.tile([128, 128], F32R, tag="oTn")
                        nc.vector.tensor_tensor(oTn[:], oT[:, tsl], bcs[:, tsl],
                                                mybir.AluOpType.mult)
                        for oc in range(D // 512):
                            owsl = slice(oc * 512, (oc + 1) * 512)
                            psf = fc_ps.tile([128, 512], F32, tag="fcp")
                            nc.tensor.matmul(psf[:], oTn[:], woT[:, owsl],
                                             start=True, stop=True)
                            fcs = ph2.tile([128, 512], F32, tag="fcs")
                            nc.vector.tensor_copy(fcs[:], psf[:])
                            nc.sync.dma_start(
                                out_d[tt * 128:(tt + 1) * 128, owsl], fcs[:])

    nc.compile()
    return nc


_NC = None


def _get_nc():
    global _NC
    if _NC is None:
        _NC = build_nc()
    return _NC


def kernel(**inputs):
    x = np.ascontiguousarray(np.asarray(inputs["x"], dtype=np.float32).reshape(NTOK, D))
    Wq = np.asarray(inputs["Wq"], dtype=np.float32)
    Wk = np.asarray(inputs["Wk"], dtype=np.float32)
    Wv = np.asarray(inputs["Wv"], dtype=np.float32)
    Wo = np.asarray(inputs["Wo"], dtype=np.float32)
    bq = np.asarray(inputs["bq"], dtype=np.float32)
    bk = np.asarray(inputs["bk"], dtype=np.float32)
    bv = np.asarray(inputs["bv"], dtype=np.float32)
    bo = np.asarray(inputs["bo"], dtype=np.float32)

    nc = _get_nc()
    in_maps = []
    for c in range(NCORES):
        sl = slice(c * OSL, (c + 1) * OSL)
        in_maps.append({
            "x": x,
            "wq": np.ascontiguousarray(Wq[sl, :]),
            "wk": np.ascontiguousarray(Wk[sl, :]),
            "wv": np.ascontiguousarray(Wv[sl, :]),
            "bq": np.ascontiguousarray(bq[sl]),
            "bk": np.ascontiguousarray(bk[sl]),
            "bv": np.ascontiguousarray(bv[sl]),
            "wo": np.ascontiguousarray(Wo[:, sl]),
        })
    res = run_bass_kernel_spmd(nc, in_maps, core_ids=list(range(NCORES)))
    acc = np.zeros((NTOK, D), dtype=np.float64)
    for c in range(NCORES):
        acc += res.results[c]["out"]
    acc += bo.astype(np.float64)[None, :]
    return acc.astype(np.float32).reshape(B, T, D)


# revision 13
# speedup vs baseline: 1.4552x; 1.0813x over previous
"""Trainium2 Bass kernel for MHA (B=2, T=2048, D=1024, H=16, HD=64).

Sharding: tensor-parallel over heads. Each of 8 cores handles 2 heads
(a 128-row slice of Wq/Wk/Wv, 128-column slice of Wo), for both batch
elements:
  - QKV: x is PE-transposed to x^T once, then Q^T/K^T/V^T [128, 4096]
    (head dims on partitions) via per-core weight slices.
  - Attention per batch (flash-style, no max subtraction -- scores are
    O(1) by construction): S^T[k,q] tiles with d=64 contraction, the two
    heads row-packed on the PE array; exp on ScalarE (softmax scale
    fused into the activation's free affine); PV with an extra ones
    column in V so the softmax denominator falls out of the same
    matmuls.
  - Softmax division commutes with fc_out, so it is deferred: fc_out is
    split per head (row-packed K=64 matmul pairs), each half scaled by
    its head's per-token reciprocal (a per-partition tensor_scalar),
    halves summed on GpSimd.
  - The 8 partial outputs are summed on the host (the all-reduce of the
    row-sharded fc_out happens at gather time); bo is added on host.

All matmuls run in float32r (TF32-like, ~1e-4 rel err, 4x faster than
fp32 on the PE); accumulation is fp32 in PSUM.
"""

import sys

sys.path.insert(0, "/opt/trn_rl_repo")

import numpy as np

import concourse.bass as bass
import concourse.mybir as mybir
import concourse.tile as tile
from concourse import bacc
from concourse.bass_utils import run_bass_kernel_spmd
from concourse.masks import make_identity

DT = mybir.dt
AF = mybir.ActivationFunctionType

B, T, D, H, HD = 2, 2048, 1024, 16, 64
NTOK = B * T              # 4096
NCORES = 8
OSL = D // NCORES         # 128 output dims per core (2 heads)
TCH = 512                 # QKV token chunk
NCH = NTOK // TCH         # 8
QC = 512                  # attention q chunk
KTILES = T // 128         # 16 k tiles per batch
SCALE = 1.0 / np.sqrt(HD)

F32 = DT.float32
F32R = DT.float32r


def build_nc():
    nc = bacc.Bacc("TRN2", target_bir_lowering=False, debug=False)

    x_d = nc.dram_tensor("x", [NTOK, D], F32, kind="ExternalInput")
    wq_d = nc.dram_tensor("wq", [OSL, D], F32, kind="ExternalInput")
    wk_d = nc.dram_tensor("wk", [OSL, D], F32, kind="ExternalInput")
    wv_d = nc.dram_tensor("wv", [OSL, D], F32, kind="ExternalInput")
    bq_d = nc.dram_tensor("bq", [OSL], F32, kind="ExternalInput")
    bk_d = nc.dram_tensor("bk", [OSL], F32, kind="ExternalInput")
    bv_d = nc.dram_tensor("bv", [OSL], F32, kind="ExternalInput")
    wo_d = nc.dram_tensor("wo", [D, OSL], F32, kind="ExternalInput")
    out_d = nc.dram_tensor("out", [NTOK, D], F32, kind="ExternalOutput")

    with tile.TileContext(nc) as tc:
        with tc.tile_pool(name="persist", bufs=1) as pp:
            # ---- constants ----
            ident = pp.tile([128, 128], F32, tag="ident")
            make_identity(nc, ident[:])
            identr = pp.tile([128, 128], F32R, tag="identr")
            nc.vector.tensor_copy(identr[:], ident[:])

            # ---- weights prep: W slices -> transposed fp32r tiles ----
            wt = {}
            with tc.tile_pool(name="prep", bufs=2) as prep, \
                 tc.tile_pool(name="prep_ps", bufs=2, space="PSUM") as prep_ps:
                for name, w_d in (("q", wq_d), ("k", wk_d), ("v", wv_d)):
                    w_nat = prep.tile([128, D], F32, tag="wnat")
                    nc.sync.dma_start(w_nat[:], w_d[:, :])
                    wt_t = pp.tile([128, 8, 128], F32R, tag=f"wt_{name}")
                    for it in range(8):
                        ps = prep_ps.tile([128, 128], F32, tag="wps")
                        nc.tensor.transpose(ps[:], w_nat[:, it * 128:(it + 1) * 128], ident[:])
                        nc.vector.tensor_copy(wt_t[:, it, :], ps[:])
                    wt[name] = wt_t
                # Wo slice [D, OSL]: natural [o2-part, d] -> WoT [d-part, o2]
                wo_nat = prep.tile([128, 8, OSL], F32, tag="wonat")
                nc.sync.dma_start(wo_nat[:], wo_d.rearrange("(a p) d -> p a d", p=128))
                woT = pp.tile([128, D], F32R, tag="woT")
                for it in range(8):
                    ps = prep_ps.tile([128, 128], F32, tag="wps")
                    nc.tensor.transpose(ps[:], wo_nat[:, it, :], ident[:])
                    nc.vector.tensor_copy(woT[:, it * 128:(it + 1) * 128], ps[:])

            btiles = {}
            for name, b_d in (("q", bq_d), ("k", bk_d), ("v", bv_d)):
                bt = pp.tile([128, 1], F32, tag=f"b_{name}")
                nc.sync.dma_start(bt[:], b_d[:, None])
                btiles[name] = bt

            # ---- persistent activations ----
            qT = pp.tile([128, NTOK], F32R, tag="qT")
            kT = pp.tile([128, NTOK], F32R, tag="kT")
            # V natural with ones column per t-tile: 32 slots of [128, 130]
            vaug = pp.tile([128, 32 * 130], F32R, tag="vaug")
            oT = pp.tile([128, NTOK], F32R, tag="oT")
            # per-token softmax reciprocal, token-partition layout, per head
            recipA = pp.tile([128, 32], F32, tag="recipA")
            recipB = pp.tile([128, 32], F32, tag="recipB")
            # reciprocal broadcast across head-dim partitions [d, q]
            bcs = pp.tile([128, NTOK], F32, tag="bcs")

            # ones columns of vaug (col 64 = head A, col 129 = head B);
            # memset doesn't support fp32r, so copy from an fp32 ones tile
            ones2 = pp.tile([128, 2], F32, tag="ones2")
            nc.vector.memset(ones2[:], 1.0)
            for ti in range(32):
                nc.vector.tensor_copy(
                    vaug[:, ti * 130 + 64:ti * 130 + 130:65], ones2[:])

            # ================= Phase 1: QKV =================
            with tc.tile_pool(name="ph1", bufs=2) as ph1, \
                 tc.tile_pool(name="ph1_ps", bufs=2, space="PSUM") as ph1_ps, \
                 tc.tile_pool(name="ph1_ps3", bufs=3, space="PSUM") as ph1_ps3:
                for ch in range(NCH):
                    t0 = ch * TCH
                    x_nat = ph1.tile([128, 4, D], F32R, tag="xnat")
                    nc.gpsimd.dma_start(
                        x_nat[:], x_d[t0:t0 + TCH, :].rearrange("(a p) i -> p a i", p=128))
                    xT = ph1.tile([128, 8, TCH], F32R, tag="xT")
                    for it in range(8):
                        ps = ph1_ps.tile([128, 512], F32R, tag="xtp")
                        for a in range(4):
                            nc.tensor.transpose(
                                ps[:, a * 128:(a + 1) * 128],
                                x_nat[:, a, it * 128:(it + 1) * 128], identr[:])
                        nc.scalar.copy(xT[:, it, :], ps[:])
                    for name in ("q", "k", "v"):
                        psq = ph1_ps3.tile([128, TCH], F32, tag="qkvp")
                        for it in range(8):
                            nc.tensor.matmul(
                                psq[:], wt[name][:, it, :], xT[:, it, :],
                                start=(it == 0), stop=(it == 7))
                        if name == "q":
                            nc.vector.tensor_scalar_add(qT[:, t0:t0 + TCH], psq[:], btiles[name][:])
                        elif name == "k":
                            nc.vector.tensor_scalar_add(kT[:, t0:t0 + TCH], psq[:], btiles[name][:])
                        else:
                            vT = ph1.tile([128, TCH], F32R, tag="vT")
                            nc.vector.tensor_scalar_add(vT[:], psq[:], btiles[name][:])
                            # transpose V^T [d, t] -> V natural [t, d] into vaug
                            psv = ph1_ps.tile([128, 4, 128], F32R, tag="vtp")
                            for a in range(4):
                                nc.tensor.transpose(
                                    psv[:, a, :], vT[:, a * 128:(a + 1) * 128], identr[:])
                            ti0 = ch * 4  # first of the 4 t-tiles in this chunk
                            va_view = vaug[:, ti0 * 130:(ti0 + 4) * 130].rearrange(
                                "p (a c) -> p a c", c=130)
                            nc.vector.tensor_copy(va_view[:, :, 0:64], psv[:, :, 0:64])
                            nc.vector.tensor_copy(va_view[:, :, 65:129], psv[:, :, 64:128])

            # ======== Phase 2: attention + fc_out, per batch ========
            # q-chunks of 512; S^T for both heads lives in ONE [128, 1024]
            # psum tile (head A cols 0:512, head B cols 512:1024) so the
            # row-packed S matmul pair shares a single wait and the exp
            # covers both heads in one F=1024 activation. fc_out for batch
            # b overlaps attention for batch b+1 (8 PSUM banks total).
            with tc.tile_pool(name="ph2", bufs=3) as ph2, \
                 tc.tile_pool(name="att_ps", bufs=1, space="PSUM") as att_ps, \
                 tc.tile_pool(name="st_ps", bufs=2, space="PSUM") as st_ps, \
                 tc.tile_pool(name="fc_ps", bufs=2, space="PSUM") as fc_ps, \
                 tc.tile_pool(name="dramp", bufs=2, space="DRAM") as dramp:
                for b in range(B):
                    for qc in range(T // QC):
                        q0 = b * T + qc * QC
                        otA = att_ps.tile([65, QC], F32, tag="otA")
                        otB = att_ps.tile([65, QC], F32, tag="otB")
                        def emit_pv(kt, pTAB):
                            ti = (b * T + kt * 128) // 128
                            nc.tensor.matmul(
                                otA[:, :], vaug[:, ti * 130 + 0:ti * 130 + 65],
                                pTAB[:, 0:QC],
                                start=(kt == 0), stop=(kt == KTILES - 1))
                            nc.tensor.matmul(
                                otB[:, :], vaug[:, ti * 130 + 65:ti * 130 + 130],
                                pTAB[:, QC:2 * QC],
                                start=(kt == 0), stop=(kt == KTILES - 1))

                        # software pipeline: S/exp(kt) emitted before PV(kt-1)
                        # so the PE prioritizes feeding ScalarE's exp stream
                        prev = None
                        for kt in range(KTILES):
                            k0 = b * T + kt * 128
                            stAB = st_ps.tile([128, 2 * QC], F32, tag="stAB")
                            nc.tensor.matmul(
                                stAB[:, 0:QC], kT[0:64, k0:k0 + 128],
                                qT[0:64, q0:q0 + QC],
                                start=True, stop=True, tile_position=(0, 0))
                            nc.tensor.matmul(
                                stAB[:, QC:2 * QC], kT[64:128, k0:k0 + 128],
                                qT[64:128, q0:q0 + QC],
                                start=True, stop=True, tile_position=(64, 0))
                            pTAB = ph2.tile([128, 2 * QC], F32R, tag="pTAB")
                            nc.scalar.activation(pTAB[:], stAB[:], AF.Exp, scale=float(SCALE))
                            if prev is not None:
                                emit_pv(kt - 1, prev)
                            prev = pTAB
                        emit_pv(KTILES - 1, prev)
                        # unnormalized head outputs -> SBUF
                        nc.vector.tensor_copy(oT[0:64, q0:q0 + QC], otA[0:64, :])
                        nc.vector.tensor_copy(oT[64:128, q0:q0 + QC], otB[0:64, :])
                        # softmax denominators (psum row 64) -> SBUF rows
                        sumsA = ph2.tile([1, QC], F32, tag="sumsA")
                        sumsB = ph2.tile([1, QC], F32, tag="sumsB")
                        nc.vector.tensor_copy(sumsA[:], otA[64:65, :])
                        nc.vector.tensor_copy(sumsB[:], otB[64:65, :])
                        # transpose to token-partition layout, reciprocal
                        tt0 = q0 // 128
                        rcA = att_ps.tile([128, QC // 128], F32, tag="otA")
                        rcB = att_ps.tile([128, QC // 128], F32, tag="otB")
                        for i in range(QC // 128):
                            nc.tensor.transpose(
                                rcA[:, i:i + 1], sumsA[0:1, i * 128:(i + 1) * 128],
                                ident[0:1, 0:1])
                            nc.tensor.transpose(
                                rcB[:, i:i + 1], sumsB[0:1, i * 128:(i + 1) * 128],
                                ident[0:1, 0:1])
                        nc.vector.reciprocal(recipA[:, tt0:tt0 + QC // 128], rcA[:])
                        nc.vector.reciprocal(recipB[:, tt0:tt0 + QC // 128], rcB[:])

                    # -- batch b reciprocal broadcast: recip [128, 16] -> PE
                    # transpose -> [16, 128] -> DRAM row -> partition-broadcast
                    # DMA into bcs rows (per head) --
                    bsl = slice(b * T, (b + 1) * T)
                    for hname, rc8, psl in (("A", recipA, slice(0, 64)),
                                            ("B", recipB, slice(64, 128))):
                        rps = att_ps.tile([16, 128], F32, tag="otA")
                        nc.tensor.transpose(rps[:], rc8[:, b * 16:(b + 1) * 16], ident[:])
                        rb = ph2.tile([16, 128], F32, tag="rb")
                        nc.vector.tensor_copy(rb[:], rps[:])
                        row_d = dramp.tile([1, T], F32, tag=f"row{hname}")
                        nc.sync.dma_start(row_d[:], rb[:])
                        nc.sync.dma_start(bcs[psl, bsl],
                                          row_d[0:1, :].to_broadcast([64, T]))

                    # -- fc_out for batch b --
                    for tt in range(b * 16, (b + 1) * 16):
                        tsl = slice(tt * 128, (tt + 1) * 128)
                        oTn = ph2.tile([128, 128], F32R, tag="oTn")
                        nc.vector.tensor_tensor(oTn[:], oT[:, tsl], bcs[:, tsl],
                                                mybir.AluOpType.mult)
                        for oc in range(D // 512):
                            owsl = slice(oc * 512, (oc + 1) * 512)
                            psf = fc_ps.tile([128, 512], F32, tag="fcp")
                            nc.tensor.matmul(psf[:], oTn[:], woT[:, owsl],
                                             start=True, stop=True)
                            fcs = ph2.tile([128, 512], F32, tag="fcs")
                            nc.vector.tensor_copy(fcs[:], psf[:])
                            nc.sync.dma_start(
                                out_d[tt * 128:(tt + 1) * 128, owsl], fcs[:])

    nc.compile()
    return nc


_NC = None


def _get_nc():
    global _NC
    if _NC is None:
        _NC = build_nc()
    return _NC


def kernel(**inputs):
    x = np.ascontiguousarray(np.asarray(inputs["x"], dtype=np.float32).reshape(NTOK, D))
    Wq = np.asarray(inputs["Wq"], dtype=np.float32)
    Wk = np.asarray(inputs["Wk"], dtype=np.float32)
    Wv = np.asarray(inputs["Wv"], dtype=np.float32)
    Wo = np.asarray(inputs["Wo"], dtype=np.float32)
    bq = np.asarray(inputs["bq"], dtype=np.float32)
    bk = np.asarray(inputs["bk"], dtype=np.float32)
    bv = np.asarray(inputs["bv"], dtype=np.float32)
    bo = np.asarray(inputs["bo"], dtype=np.float32)

    nc = _get_nc()
    in_maps = []
    for c in range(NCORES):
        sl = slice(c * OSL, (c + 1) * OSL)
        in_maps.append({
            "x": x,
            "wq": np.ascontiguousarray(Wq[sl, :]),
            "wk": np.ascontiguousarray(Wk[sl, :]),
            "wv": np.ascontiguousarray(Wv[sl, :]),
            "bq": np.ascontiguousarray(bq[sl]),
            "bk": np.ascontiguousarray(bk[sl]),
            "bv": np.ascontiguousarray(bv[sl]),
            "wo": np.ascontiguousarray(Wo[:, sl]),
        })
    res = run_bass_kernel_spmd(nc, in_maps, core_ids=list(range(NCORES)))
    acc = np.zeros((NTOK, D), dtype=np.float64)
    for c in range(NCORES):
        acc += res.results[c]["out"]
    acc += bo.astype(np.float64)[None, :]
    return acc.astype(np.float32).reshape(B, T, D)


# revision 15
# speedup vs baseline: 1.5193x; 1.0440x over previous
"""Trainium2 Bass kernel for MHA (B=2, T=2048, D=1024, H=16, HD=64).

Sharding: tensor-parallel over heads. Each of 8 cores handles 2 heads
(a 128-row slice of Wq/Wk/Wv, 128-column slice of Wo), for both batch
elements:
  - QKV: x is PE-transposed to x^T once, then Q^T/K^T/V^T [128, 4096]
    (head dims on partitions) via per-core weight slices.
  - Attention per batch (flash-style, no max subtraction -- scores are
    O(1) by construction): S^T[k,q] tiles with d=64 contraction, the two
    heads row-packed on the PE array; exp on ScalarE (softmax scale
    fused into the activation's free affine); PV with an extra ones
    column in V so the softmax denominator falls out of the same
    matmuls.
  - Softmax division commutes with fc_out, so it is deferred: fc_out is
    split per head (row-packed K=64 matmul pairs), each half scaled by
    its head's per-token reciprocal (a per-partition tensor_scalar),
    halves summed on GpSimd.
  - The 8 partial outputs are summed on the host (the all-reduce of the
    row-sharded fc_out happens at gather time); bo is added on host.

All matmuls run in float32r (TF32-like, ~1e-4 rel err, 4x faster than
fp32 on the PE); accumulation is fp32 in PSUM.
"""

import sys

sys.path.insert(0, "/opt/trn_rl_repo")

import numpy as np

import concourse.bass as bass
import concourse.mybir as mybir
import concourse.tile as tile
from concourse import bacc
from concourse.bass_utils import run_bass_kernel_spmd
from concourse.masks import make_identity

DT = mybir.dt
AF = mybir.ActivationFunctionType

B, T, D, H, HD = 2, 2048, 1024, 16, 64
NTOK = B * T              # 4096
NCORES = 8
OSL = D // NCORES         # 128 output dims per core (2 heads)
TCH = 512                 # QKV token chunk
NCH = NTOK // TCH         # 8
QC = 512                  # attention q chunk
KTILES = T // 128         # 16 k tiles per batch
SCALE = 1.0 / np.sqrt(HD)

F32 = DT.float32
F32R = DT.float32r


def build_nc():
    nc = bacc.Bacc("TRN2", target_bir_lowering=False, debug=False)

    x_d = nc.dram_tensor("x", [NTOK, D], F32, kind="ExternalInput")
    wq_d = nc.dram_tensor("wq", [OSL, D], F32, kind="ExternalInput")
    wk_d = nc.dram_tensor("wk", [OSL, D], F32, kind="ExternalInput")
    wv_d = nc.dram_tensor("wv", [OSL, D], F32, kind="ExternalInput")
    bq_d = nc.dram_tensor("bq", [OSL], F32, kind="ExternalInput")
    bk_d = nc.dram_tensor("bk", [OSL], F32, kind="ExternalInput")
    bv_d = nc.dram_tensor("bv", [OSL], F32, kind="ExternalInput")
    wo_d = nc.dram_tensor("wo", [D, OSL], F32, kind="ExternalInput")
    out_d = nc.dram_tensor("out", [NTOK, D], F32, kind="ExternalOutput")

    with tile.TileContext(nc) as tc:
        with tc.tile_pool(name="persist", bufs=1) as pp:
            # ---- constants ----
            ident = pp.tile([128, 128], F32, tag="ident")
            make_identity(nc, ident[:])
            identr = pp.tile([128, 128], F32R, tag="identr")
            nc.vector.tensor_copy(identr[:], ident[:])

            # ---- weights prep: W slices -> transposed fp32r tiles ----
            wt = {}
            with tc.tile_pool(name="prep", bufs=2) as prep, \
                 tc.tile_pool(name="prep_ps", bufs=2, space="PSUM") as prep_ps:
                for name, w_d in (("q", wq_d), ("k", wk_d), ("v", wv_d)):
                    w_nat = prep.tile([128, D], F32, tag="wnat")
                    nc.sync.dma_start(w_nat[:], w_d[:, :])
                    wt_t = pp.tile([128, 8, 128], F32R, tag=f"wt_{name}")
                    for it in range(8):
                        ps = prep_ps.tile([128, 128], F32, tag="wps")
                        nc.tensor.transpose(ps[:], w_nat[:, it * 128:(it + 1) * 128], ident[:])
                        nc.vector.tensor_copy(wt_t[:, it, :], ps[:])
                    wt[name] = wt_t
                # Wo slice [D, OSL]: natural [o2-part, d] -> WoT [d-part, o2]
                wo_nat = prep.tile([128, 8, OSL], F32, tag="wonat")
                nc.sync.dma_start(wo_nat[:], wo_d.rearrange("(a p) d -> p a d", p=128))
                woT = pp.tile([128, D], F32R, tag="woT")
                for it in range(8):
                    ps = prep_ps.tile([128, 128], F32, tag="wps")
                    nc.tensor.transpose(ps[:], wo_nat[:, it, :], ident[:])
                    nc.vector.tensor_copy(woT[:, it * 128:(it + 1) * 128], ps[:])

            btiles = {}
            for name, b_d in (("q", bq_d), ("k", bk_d), ("v", bv_d)):
                bt = pp.tile([128, 1], F32, tag=f"b_{name}")
                nc.sync.dma_start(bt[:], b_d[:, None])
                btiles[name] = bt

            # ---- persistent activations ----
            qT = pp.tile([128, NTOK], F32R, tag="qT")
            kT = pp.tile([128, NTOK], F32R, tag="kT")
            # V natural with ones column per t-tile: 32 slots of [128, 130]
            vaug = pp.tile([128, 32 * 130], F32R, tag="vaug")
            oT = pp.tile([128, NTOK], F32R, tag="oT")
            # per-token softmax reciprocal, token-partition layout, per head
            recipA = pp.tile([128, 32], F32, tag="recipA")
            recipB = pp.tile([128, 32], F32, tag="recipB")
            # reciprocal broadcast across head-dim partitions [d, q]
            bcs = pp.tile([128, NTOK], F32, tag="bcs")

            # ones columns of vaug (col 64 = head A, col 129 = head B);
            # memset doesn't support fp32r, so copy from an fp32 ones tile
            ones2 = pp.tile([128, 2], F32, tag="ones2")
            nc.vector.memset(ones2[:], 1.0)
            for ti in range(32):
                nc.vector.tensor_copy(
                    vaug[:, ti * 130 + 64:ti * 130 + 130:65], ones2[:])

            # ================= Phase 1: QKV =================
            with tc.tile_pool(name="ph1", bufs=2) as ph1, \
                 tc.tile_pool(name="ph1_ps", bufs=2, space="PSUM") as ph1_ps, \
                 tc.tile_pool(name="ph1_ps3", bufs=3, space="PSUM") as ph1_ps3:
                for ch in range(NCH):
                    t0 = ch * TCH
                    x_nat = ph1.tile([128, 4, D], F32R, tag="xnat")
                    nc.gpsimd.dma_start(
                        x_nat[:], x_d[t0:t0 + TCH, :].rearrange("(a p) i -> p a i", p=128))
                    xT = ph1.tile([128, 8, TCH], F32R, tag="xT")
                    for it in range(8):
                        ps = ph1_ps.tile([128, 512], F32R, tag="xtp")
                        for a in range(4):
                            nc.tensor.transpose(
                                ps[:, a * 128:(a + 1) * 128],
                                x_nat[:, a, it * 128:(it + 1) * 128], identr[:])
                        nc.scalar.copy(xT[:, it, :], ps[:])
                    for name in ("q", "k", "v"):
                        psq = ph1_ps3.tile([128, TCH], F32, tag="qkvp")
                        for it in range(8):
                            nc.tensor.matmul(
                                psq[:], wt[name][:, it, :], xT[:, it, :],
                                start=(it == 0), stop=(it == 7))
                        if name == "q":
                            nc.vector.tensor_scalar_add(qT[:, t0:t0 + TCH], psq[:], btiles[name][:])
                        elif name == "k":
                            nc.vector.tensor_scalar_add(kT[:, t0:t0 + TCH], psq[:], btiles[name][:])
                        else:
                            vT = ph1.tile([128, TCH], F32R, tag="vT")
                            nc.vector.tensor_scalar_add(vT[:], psq[:], btiles[name][:])
                            # transpose V^T [d, t] -> V natural [t, d] into vaug
                            psv = ph1_ps.tile([128, 4, 128], F32R, tag="vtp")
                            for a in range(4):
                                nc.tensor.transpose(
                                    psv[:, a, :], vT[:, a * 128:(a + 1) * 128], identr[:])
                            ti0 = ch * 4  # first of the 4 t-tiles in this chunk
                            va_view = vaug[:, ti0 * 130:(ti0 + 4) * 130].rearrange(
                                "p (a c) -> p a c", c=130)
                            nc.vector.tensor_copy(va_view[:, :, 0:64], psv[:, :, 0:64])
                            nc.vector.tensor_copy(va_view[:, :, 65:129], psv[:, :, 64:128])

            # ======== Phase 2: attention + fc_out, per batch ========
            # q-chunks of 512; S^T for both heads lives in ONE [128, 1024]
            # psum tile (head A cols 0:512, head B cols 512:1024) so the
            # row-packed S matmul pair shares a single wait and the exp
            # covers both heads in one F=1024 activation. fc_out for batch
            # b overlaps attention for batch b+1 (8 PSUM banks total).
            with tc.tile_pool(name="ph2", bufs=3) as ph2, \
                 tc.tile_pool(name="att_ps", bufs=1, space="PSUM") as att_ps, \
                 tc.tile_pool(name="st_ps", bufs=2, space="PSUM") as st_ps, \
                 tc.tile_pool(name="fc_ps", bufs=2, space="PSUM") as fc_ps, \
                 tc.tile_pool(name="dramp", bufs=2, space="DRAM") as dramp:
                for b in range(B):
                    for qc in range(T // QC):
                        q0 = b * T + qc * QC
                        otA = att_ps.tile([65, QC], F32, tag="otA")
                        otB = att_ps.tile([65, QC], F32, tag="otB")
                        def emit_pv(kt, pTAB):
                            ti = (b * T + kt * 128) // 128
                            nc.tensor.matmul(
                                otA[:, :], vaug[:, ti * 130 + 0:ti * 130 + 65],
                                pTAB[:, 0:QC],
                                start=(kt == 0), stop=(kt == KTILES - 1))
                            nc.tensor.matmul(
                                otB[:, :], vaug[:, ti * 130 + 65:ti * 130 + 130],
                                pTAB[:, QC:2 * QC],
                                start=(kt == 0), stop=(kt == KTILES - 1))

                        # software pipeline: S/exp(kt) emitted before PV(kt-1)
                        # so the PE prioritizes feeding ScalarE's exp stream
                        prev = None
                        for kt in range(KTILES):
                            k0 = b * T + kt * 128
                            stAB = st_ps.tile([128, 2 * QC], F32, tag="stAB")
                            nc.tensor.matmul(
                                stAB[:, 0:QC], kT[0:64, k0:k0 + 128],
                                qT[0:64, q0:q0 + QC],
                                start=True, stop=True, tile_position=(0, 0))
                            nc.tensor.matmul(
                                stAB[:, QC:2 * QC], kT[64:128, k0:k0 + 128],
                                qT[64:128, q0:q0 + QC],
                                start=True, stop=True, tile_position=(64, 0))
                            pTAB = ph2.tile([128, 2 * QC], F32R, tag="pTAB")
                            nc.scalar.activation(pTAB[:], stAB[:], AF.Exp, scale=float(SCALE))
                            if prev is not None:
                                emit_pv(kt - 1, prev)
                            prev = pTAB
                        emit_pv(KTILES - 1, prev)
                        # unnormalized head outputs -> SBUF
                        nc.vector.tensor_copy(oT[0:64, q0:q0 + QC], otA[0:64, :])
                        nc.vector.tensor_copy(oT[64:128, q0:q0 + QC], otB[0:64, :])
                        # softmax denominators (psum row 64) -> SBUF rows
                        sumsA = ph2.tile([1, QC], F32, tag="sumsA")
                        sumsB = ph2.tile([1, QC], F32, tag="sumsB")
                        nc.vector.tensor_copy(sumsA[:], otA[64:65, :])
                        nc.vector.tensor_copy(sumsB[:], otB[64:65, :])
                        # transpose to token-partition layout, reciprocal
                        tt0 = q0 // 128
                        rcA = att_ps.tile([128, QC // 128], F32, tag="otA")
                        rcB = att_ps.tile([128, QC // 128], F32, tag="otB")
                        for i in range(QC // 128):
                            nc.tensor.transpose(
                                rcA[:, i:i + 1], sumsA[0:1, i * 128:(i + 1) * 128],
                                ident[0:1, 0:1])
                            nc.tensor.transpose(
                                rcB[:, i:i + 1], sumsB[0:1, i * 128:(i + 1) * 128],
                                ident[0:1, 0:1])
                        nc.vector.reciprocal(recipA[:, tt0:tt0 + QC // 128], rcA[:])
                        nc.vector.reciprocal(recipB[:, tt0:tt0 + QC // 128], rcB[:])

                        # -- q-chunk reciprocal broadcast: recip [128, 4] -> PE
                        # transpose -> [4, 128] -> DRAM row -> partition-
                        # broadcast DMA into bcs rows (per head) --
                        qsl = slice(q0, q0 + QC)
                        nq = QC // 128
                        for hname, rc8, psl in (("A", recipA, slice(0, 64)),
                                                ("B", recipB, slice(64, 128))):
                            rps = att_ps.tile([nq, 128], F32, tag="otA")
                            nc.tensor.transpose(rps[:], rc8[:, tt0:tt0 + nq], ident[:])
                            rb = ph2.tile([nq, 128], F32, tag="rb")
                            nc.vector.tensor_copy(rb[:], rps[:])
                            row_d = dramp.tile([1, QC], F32, tag=f"row{hname}")
                            nc.sync.dma_start(row_d[:], rb[:])
                            nc.sync.dma_start(bcs[psl, qsl],
                                              row_d[0:1, :].to_broadcast([64, QC]))

                        # -- fc_out for this q-chunk --
                        for tt in range(tt0, tt0 + nq):
                            tsl = slice(tt * 128, (tt + 1) * 128)
                            oTn = ph2.tile([128, 128], F32R, tag="oTn")
                            nc.vector.tensor_tensor(oTn[:], oT[:, tsl], bcs[:, tsl],
                                                    mybir.AluOpType.mult)
                            for oc in range(D // 512):
                                owsl = slice(oc * 512, (oc + 1) * 512)
                                psf = fc_ps.tile([128, 512], F32, tag="fcp")
                                nc.tensor.matmul(psf[:], oTn[:], woT[:, owsl],
                                                 start=True, stop=True)
                                fcs = ph2.tile([128, 512], F32, tag="fcs")
                                nc.vector.tensor_copy(fcs[:], psf[:])
                                nc.sync.dma_start(
                                    out_d[tt * 128:(tt + 1) * 128, owsl], fcs[:])

    nc.compile()
    return nc


_NC = None


def _get_nc():
    global _NC
    if _NC is None:
        _NC = build_nc()
    return _NC


def kernel(**inputs):
    x = np.ascontiguousarray(np.asarray(inputs["x"], dtype=np.float32).reshape(NTOK, D))
    Wq = np.asarray(inputs["Wq"], dtype=np.float32)
    Wk = np.asarray(inputs["Wk"], dtype=np.float32)
    Wv = np.asarray(inputs["Wv"], dtype=np.float32)
    Wo = np.asarray(inputs["Wo"], dtype=np.float32)
    bq = np.asarray(inputs["bq"], dtype=np.float32)
    bk = np.asarray(inputs["bk"], dtype=np.float32)
    bv = np.asarray(inputs["bv"], dtype=np.float32)
    bo = np.asarray(inputs["bo"], dtype=np.float32)

    nc = _get_nc()
    in_maps = []
    for c in range(NCORES):
        sl = slice(c * OSL, (c + 1) * OSL)
        in_maps.append({
            "x": x,
            "wq": np.ascontiguousarray(Wq[sl, :]),
            "wk": np.ascontiguousarray(Wk[sl, :]),
            "wv": np.ascontiguousarray(Wv[sl, :]),
            "bq": np.ascontiguousarray(bq[sl]),
            "bk": np.ascontiguousarray(bk[sl]),
            "bv": np.ascontiguousarray(bv[sl]),
            "wo": np.ascontiguousarray(Wo[:, sl]),
        })
    res = run_bass_kernel_spmd(nc, in_maps, core_ids=list(range(NCORES)))
    acc = np.zeros((NTOK, D), dtype=np.float64)
    for c in range(NCORES):
        acc += res.results[c]["out"]
    acc += bo.astype(np.float64)[None, :]
    return acc.astype(np.float32).reshape(B, T, D)
